# revision 10
# baseline (speedup 1.0000x reference)
"""BDC loss kernel for 8 Trainium2 NeuronCores.

reference:
    intra = mean over rows of ||f - c_l||^2 / exp(cos(f, c_l))
    adv   = sum over label-differing ordered pairs of relu(0.5 - cos_sim(f_i, f_j)) / n_pairs
    out   = intra + 0.5 * adv

Strategy (SPMD, one program on 8 cores, per-core data differs):
  - The B x B cosine-sim hinge sum is symmetric; each unordered tile-pair is
    computed once using a circulant assignment over the 64 row-tiles of 128:
    global row-tile A computes col-tiles at distance d = 0..32 (mod 64).
  - HOST does all O(B*D) prep: sort rows by label, normalize features and
    per-row centers, cast to fp8e4, and transpose to K-major. The device
    receives matmul-ready operands and does only:
      * fp8 DoubleRow matmuls (PE) into 4-bank [128, 2048] PSUM tiles
      * fused hinge + row-sum evictions, one instruction per 2048 cols,
        alternating between DVE (min(x-m, 0), negated) and ACT
        (relu(m - x), positive); host fixes signs via per-slot weights
      * same-label corrections on the otherwise-idle Pool (gpsimd) engine
      * a tiny intra phase: fhat . chat per row via packed 128x128 matmuls,
        diagonal extracted by elementwise-mult-with-identity + row-accum
  - Host finishes the scalar math: intra_i = (|f|^2+|c|^2 - 2|f||c| sim_i)
    * exp(-sim_i), slot-weighted adversarial sum, n_pairs.
"""

import numpy as np
import ml_dtypes

B, D, C = 8192, 1024, 1000
NCORES = 8
SHARD = B // NCORES            # 1024 rows owned per core
RT = SHARD // 128              # 8 row-tiles per core
NTILES = B // 128              # 64 global row-tiles
DMAX = 32                      # circulant distance range 0..32
LROWS = (RT + DMAX) * 128      # 5120 local rows each core needs
KT = D // 128                  # 8 K-chunks
GCOLS = 2048                   # columns per PSUM group (4 banks)
NGRP = 2                       # groups per row-tile (2*2048 = 4096 = d0..d31)
NSLOT = 48
# fhatT DMA col-blocks: small first for fast PE start, then big (few issues
# -- DMA issue costs ~5.7ns/KB serialized on the issuing sequencer)
FBLOCKS = [128, 128, 128, 128, 256, 256, 256, 256, 512, 512, 1024, 1024, 512]
ALPHA, LAMBDA_ADV, MARGIN, EPS = 1.0, 0.5, 0.5, 1e-8

# slot layout: 0..15 group sums; 16..23 d0 reweight; 24..31 same-label d0;
# 32..39 same-label d1-strip; 40 d32 blocks
SLOT_D0ADJ, SLOT_CORR1, SLOT_CORR2, SLOT_D32 = 16, 24, 32, 40

_CACHE = {}


def _units():
    """Emission order of the 16 chunk-groups + eviction engine parity."""
    grps = sorted(
        [(rt, g) for rt in range(RT) for g in range(NGRP)],
        key=lambda u: u[0] * 128 + (u[1] + 1) * GCOLS)
    out = []
    for i, (rt, g) in enumerate(grps):
        # ACT is the cheaper evictor (1.2GHz + fused relu); DVE also carries
        # the correction ops and intra diag extracts, so give ACT 9 of 16.
        eng = "act" if (i % 2 == 0 or i == len(grps) - 1) else "dve"
        out.append((rt, g, i, eng))
    return out


def _slot_weights():
    w = np.zeros(NSLOT, dtype=np.float64)
    g0_eng = {}
    for rt, g, s, eng in _units():
        w[s] = 2.0 if eng == "act" else -2.0
        if g == 0:
            g0_eng[rt] = eng
    for rt in range(RT):
        sgn = -1.0 if g0_eng[rt] == "act" else 1.0
        w[SLOT_D0ADJ + rt] = sgn
        w[SLOT_CORR1 + rt] = sgn
        w[SLOT_CORR2 + rt] = 2.0 * sgn
    w[SLOT_D32] = 1.0
    # last unit's eviction is split: DVE half in its own slot (negated),
    # ACT half in slot NSLOT-1 (positive)
    w[NGRP * RT - 1] = -2.0
    w[NSLOT - 1] = 2.0
    return w


def _build():
    import concourse.bass as bass
    import concourse.tile as tile
    from concourse import bacc, mybir
    from concourse.masks import make_identity

    f32 = mybir.dt.float32
    f16 = mybir.dt.float16
    bf16 = mybir.dt.bfloat16
    f8 = mybir.dt.float8e4
    DR = mybir.MatmulPerfMode.DoubleRow
    Relu = mybir.ActivationFunctionType.Relu

    nc = bacc.Bacc("TRN2", target_bir_lowering=False, debug=False,
                   num_devices=NCORES)

    # host-prepped K-major operands, packed per col-block in (p, k, c)
    # C-order so each block is one contiguous-DRAM DMA with 2-4KB lines
    fhatT_dram = nc.dram_tensor("fhatT", [128 * KT * LROWS], f8,
                                kind="ExternalInput")
    chatT_dram = nc.dram_tensor("chatT", [128 * KT * SHARD], f8,
                                kind="ExternalInput")
    lab16_dram = nc.dram_tensor("lab_f16", [(RT + 2) * 128], f16,
                                kind="ExternalInput")
    labf_dram = nc.dram_tensor("lab_row", [SHARD], f16, kind="ExternalInput")
    adv_dram = nc.dram_tensor("adv_out", [128, NSLOT], f32,
                              kind="ExternalOutput")
    sim_dram = nc.dram_tensor("sim_out", [128, RT], f32,
                              kind="ExternalOutput")

    with tile.TileContext(nc) as tc:
        from contextlib import ExitStack
        with ExitStack() as ctx:
            singles = ctx.enter_context(tc.tile_pool(name="singles", bufs=1))
            work = ctx.enter_context(tc.tile_pool(name="work", bufs=4))
            scrp = ctx.enter_context(tc.tile_pool(name="scrp", bufs=2))
            psum = ctx.enter_context(
                tc.tile_pool(name="psum", bufs=2, space=bass.MemorySpace.PSUM))

            fhat_sb = singles.tile([128, KT, LROWS], f8)
            chat_sb = singles.tile([128, KT, SHARD], f8)
            labcol = singles.tile([128, (RT + 2) * 128], f16)
            labrow = singles.tile([128, RT], f16)
            ident = singles.tile([128, 128], bf16)
            adv_acc = singles.tile([128, NSLOT], f32)
            sim_col = singles.tile([128, RT], f32)

            # warm the ACT Relu table before any real dependency
            warm = singles.tile([128, 1], f32)
            nc.vector.memset(warm[:], 1.0)
            nc.scalar.activation(out=warm[:], in_=warm[:], func=Relu)
            marg = singles.tile([128, 1], f32)
            nc.vector.memset(marg[:], MARGIN)
            nc.vector.memset(adv_acc[:], 0.0)
            make_identity(nc, ident[:])

            # label tiles: labels broadcast along partitions via 0-stride DMA
            # (issued from the ACT sequencer; SP is saturated with fhat)
            nc.scalar.dma_start(
                out=labcol[:],
                in_=bass.AP(tensor=lab16_dram, offset=0,
                            ap=[[0, 128], [1, (RT + 2) * 128]]))
            nc.scalar.dma_start(
                out=labrow[:],
                in_=labf_dram.ap().rearrange("(t p) -> p t", p=128))

            # fhatT: one DMA per col-block (all 8 k-slices, all partitions);
            # descriptors of a single dma_start spread across all 16 DMA
            # engines, so few big DMAs beat many small ones. Alternate the
            # issuing sequencer (SP / ACT) to halve issue serialization.
            # chat on ACT (in parallel with SP's first fhat blocks), split
            # per row-tile so the intra warm-up unit starts ~9us in
            for rt in range(RT):
                bc = 128
                nc.scalar.dma_start(
                    out=chat_sb[:, :, rt * bc:(rt + 1) * bc],
                    in_=bass.AP(tensor=chatT_dram, offset=128 * KT * bc * rt,
                                ap=[[KT * bc, 128], [1, KT * bc]]))
            off = 0
            c0 = 0
            for bc in FBLOCKS:
                nc.sync.dma_start(
                    out=fhat_sb[:, :, c0:c0 + bc],
                    in_=bass.AP(tensor=fhatT_dram, offset=off,
                                ap=[[KT * bc, 128], [1, KT * bc]]))
                off += 128 * KT * bc
                c0 += bc

            def emit_group(rt, g, slot, eng, split=False):
                pt = psum.tile([128, GCOLS], f32)
                for ch in range(4):
                    c0 = rt * 128 + g * GCOLS + ch * 512
                    for k2 in range(KT // 2):
                        nc.tensor.matmul(
                            out=pt[:, ch * 512:(ch + 1) * 512],
                            lhsT=fhat_sb[:, 2 * k2:2 * k2 + 2,
                                         rt * 128:(rt + 1) * 128],
                            rhs=fhat_sb[:, 2 * k2:2 * k2 + 2, c0:c0 + 512],
                            perf_mode=DR,
                            start=(k2 == 0), stop=(k2 == KT // 2 - 1))
                negh = work.tile([128, GCOLS], f16, tag="negh")
                if split:
                    # tail trim: half on DVE (slot), half on ACT (slot+1)
                    nc.vector.tensor_scalar(
                        out=negh[:, 0:1024], in0=pt[:, 0:1024],
                        scalar1=MARGIN, scalar2=0.0,
                        op0=mybir.AluOpType.subtract,
                        op1=mybir.AluOpType.min,
                        accum_out=adv_acc[:, slot:slot + 1])
                    nc.scalar.activation(
                        out=negh[:, 1024:2048], in_=pt[:, 1024:2048],
                        func=Relu, scale=-1.0, bias=marg[:],
                        accum_out=adv_acc[:, NSLOT - 1:NSLOT])
                    return
                if eng == "dve":
                    # negated hinge: min(sim - margin, 0), row-sum fused
                    nc.vector.tensor_scalar(
                        out=negh[:], in0=pt[:], scalar1=MARGIN, scalar2=0.0,
                        op0=mybir.AluOpType.subtract,
                        op1=mybir.AluOpType.min,
                        accum_out=adv_acc[:, slot:slot + 1])
                else:
                    # positive hinge: relu(margin - sim), row-sum fused
                    nc.scalar.activation(
                        out=negh[:], in_=pt[:], func=Relu,
                        scale=-1.0, bias=marg[:],
                        accum_out=adv_acc[:, slot:slot + 1])
                if g == 0:
                    # d0 tile needs weight 1 not 2: re-accumulate its strip;
                    # same-label corrections live in the first 256 cols.
                    # All on the otherwise-idle Pool engine (SBUF-only).
                    s1 = scrp.tile([128, 128], f16, tag="scr")
                    nc.vector.tensor_scalar(
                        out=s1[:], in0=negh[:, 0:128], scalar1=1.0,
                        scalar2=0.0, op0=mybir.AluOpType.mult,
                        op1=mybir.AluOpType.add,
                        accum_out=adv_acc[:, SLOT_D0ADJ + rt:
                                          SLOT_D0ADJ + rt + 1])
                    s2 = scrp.tile([128, 128], f16, tag="scr")
                    nc.vector.scalar_tensor_tensor(
                        out=s2[:], in0=labcol[:, rt * 128:(rt + 1) * 128],
                        scalar=labrow[:, rt:rt + 1], in1=negh[:, 0:128],
                        op0=mybir.AluOpType.is_equal,
                        op1=mybir.AluOpType.mult,
                        accum_out=adv_acc[:, SLOT_CORR1 + rt:
                                          SLOT_CORR1 + rt + 1])
                    s3 = scrp.tile([128, 128], f16, tag="scr")
                    nc.vector.scalar_tensor_tensor(
                        out=s3[:], in0=labcol[:, (rt + 1) * 128:
                                              (rt + 2) * 128],
                        scalar=labrow[:, rt:rt + 1], in1=negh[:, 128:256],
                        op0=mybir.AluOpType.is_equal,
                        op1=mybir.AluOpType.mult,
                        accum_out=adv_acc[:, SLOT_CORR2 + rt:
                                          SLOT_CORR2 + rt + 1])

            def emit_intra():
                pt = psum.tile([128, GCOLS], f32)
                for rt in range(RT):
                    for k2 in range(KT // 2):
                        nc.tensor.matmul(
                            out=pt[:, rt * 128:(rt + 1) * 128],
                            lhsT=fhat_sb[:, 2 * k2:2 * k2 + 2,
                                         rt * 128:(rt + 1) * 128],
                            rhs=chat_sb[:, 2 * k2:2 * k2 + 2,
                                        rt * 128:(rt + 1) * 128],
                            perf_mode=DR,
                            start=(k2 == 0), stop=(k2 == KT // 2 - 1))
                for rt in range(RT):
                    s = scrp.tile([128, 128], f16, tag="scr")
                    nc.vector.scalar_tensor_tensor(
                        out=s[:], in0=pt[:, rt * 128:(rt + 1) * 128],
                        scalar=1.0, in1=ident[:],
                        op0=mybir.AluOpType.mult, op1=mybir.AluOpType.mult,
                        accum_out=sim_col[:, rt:rt + 1])

            def emit_d32():
                pt = psum.tile([128, GCOLS], f32)
                for rt in range(RT):
                    for k2 in range(KT // 2):
                        nc.tensor.matmul(
                            out=pt[:, rt * 128:(rt + 1) * 128],
                            lhsT=fhat_sb[:, 2 * k2:2 * k2 + 2,
                                         rt * 128:(rt + 1) * 128],
                            rhs=fhat_sb[:, 2 * k2:2 * k2 + 2,
                                        (rt + DMAX) * 128:
                                        (rt + DMAX + 1) * 128],
                            perf_mode=DR,
                            start=(k2 == 0), stop=(k2 == KT // 2 - 1))
                negh = work.tile([128, GCOLS], f16, tag="negh")
                nc.scalar.activation(
                    out=negh[:, 0:1024], in_=pt[:, 0:1024], func=Relu,
                    scale=-1.0, bias=marg[:],
                    accum_out=adv_acc[:, SLOT_D32:SLOT_D32 + 1])

            units = _units()
            emit_intra()
            for i, (rt, g, slot, eng) in enumerate(units):
                if i == 12:
                    emit_d32()
                emit_group(rt, g, slot, eng, split=(i == len(units) - 1))

            nc.sync.dma_start(out=adv_dram.ap(), in_=adv_acc[:])
            nc.sync.dma_start(out=sim_dram.ap(), in_=sim_col[:])

    nc.compile()
    return nc


def _get_nc():
    if "nc" not in _CACHE:
        _CACHE["nc"] = _build()
    return _CACHE["nc"]


def _roll_cols(arr, s, n):
    """arr[..., (s + arange(n)) % N] via at most two contiguous copies."""
    N = arr.shape[-1]
    s = s % N
    if s + n <= N:
        return np.ascontiguousarray(arr[..., s:s + n])
    return np.ascontiguousarray(
        np.concatenate([arr[..., s:], arr[..., :s + n - N]], axis=-1))


def _make_in_maps(features, labels, centers):
    features = np.asarray(features, dtype=np.float32)
    labels = np.asarray(labels).astype(np.int64)
    centers = np.asarray(centers, dtype=np.float32)

    # loss is invariant to batch permutation: sort by label so same-label
    # pairs land within ~30 rows of the diagonal
    perm = np.argsort(labels, kind="stable")
    feat_s = features[perm]
    labels_s = labels[perm]

    fnorm = np.maximum(np.sqrt(np.einsum("ij,ij->i", feat_s, feat_s)), EPS)
    fhat8 = (feat_s / fnorm[:, None]).astype(ml_dtypes.float8_e4m3)
    # K-major global: [k*128+p, r]
    fhatT_g = np.ascontiguousarray(fhat8.T)                    # [D, B]

    def pack_blocks(kmaj, blocks):
        # kmaj: [D, N] k-major -> flat (p, k, c)-C-order per col-block
        kpc = kmaj.reshape(KT, 128, kmaj.shape[1])
        parts, c0 = [], 0
        for bc in blocks:
            parts.append(
                np.ascontiguousarray(
                    kpc[:, :, c0:c0 + bc].transpose(1, 0, 2)).reshape(-1))
            c0 += bc
        return np.concatenate(parts)

    cnorm_c = np.maximum(np.sqrt(np.einsum("ij,ij->i", centers, centers)),
                         EPS)
    chat = centers / cnorm_c[:, None]
    chat8_rows = chat[labels_s].astype(ml_dtypes.float8_e4m3)  # [B, D]
    chatT_g = np.ascontiguousarray(chat8_rows.T)               # [D, B]
    cnorm = cnorm_c[labels_s]

    lab16 = labels_s.astype(np.float16)

    in_maps = []
    for c in range(NCORES):
        s = c * SHARD
        in_maps.append({
            "fhatT": pack_blocks(_roll_cols(fhatT_g, s, LROWS), FBLOCKS),
            "chatT": pack_blocks(chatT_g[:, s:s + SHARD], [128] * RT),
            "lab_f16": _roll_cols(lab16, s, (RT + 2) * 128),
            "lab_row": np.ascontiguousarray(lab16[s:s + SHARD]),
        })
    host_scal = {"fnorm": fnorm, "cnorm": cnorm, "labels_s": labels_s}
    return in_maps, host_scal


def _combine(results, host_scal):
    w = _slot_weights()
    hinge_total = 0.0
    sims = np.zeros(B, dtype=np.float64)
    for c in range(NCORES):
        adv = results[c]["adv_out"].astype(np.float64)   # [128, NSLOT]
        hinge_total += float(adv.sum(axis=0) @ w)
        # sim_out[p, t] = sim of sorted row c*1024 + t*128 + p
        sims[c * SHARD:(c + 1) * SHARD] = \
            results[c]["sim_out"].astype(np.float64).T.reshape(SHARD)

    fn = host_scal["fnorm"].astype(np.float64)
    cn = host_scal["cnorm"].astype(np.float64)
    sq_err = fn * fn + cn * cn - 2.0 * fn * cn * sims
    intra_total = float(np.sum(sq_err * np.exp(-ALPHA * sims)))

    cnt = np.bincount(host_scal["labels_s"], minlength=C).astype(np.float64)
    n_pairs = float(B) * B - float((cnt * cnt).sum())
    n_pairs = max(n_pairs, 1.0)
    loss = intra_total / B + LAMBDA_ADV * (hinge_total / n_pairs)
    return np.float32(loss)


def kernel(features, labels, centers):
    from concourse.bass_utils import run_bass_kernel_spmd
    nc = _get_nc()
    in_maps, host_scal = _make_in_maps(features, labels, centers)
    res = run_bass_kernel_spmd(nc, in_maps, core_ids=list(range(NCORES)))
    return _combine(res.results, host_scal)
